# revision 82
# baseline (speedup 1.0000x reference)
"""Distributed Trainium2 Bass kernel for nn_AttentionCell (B=1, S=4096, D=1024, H=16).

Sharding: tensor-parallel over heads, 2 heads per core, paired (h, h+8) so RoPE's
rotate-half (which pairs model dims d and d+512, i.e. heads h and h+8) stays local
to a core. Per core:
  - QKV projection for its 128 channels (computed transposed: [ch, S]) from a
    host-staged transposed bf16 copy of x. Projection of chunk ch+1 is
    interleaved into the attention groups of q-block ch so RoPE latency and
    the ACT/PE mix stay pipelined.
  - RoPE via host-staged cos/sin tables fused with the bias-add on the
    PSUM->SBUF drain.
  - Causal attention with scores computed transposed ([k, q] layout) so the
    PV matmul needs no on-chip transposes; softmax without max-subtraction
    (scores are O(1) here, exp cannot overflow); denominator via a ones-column
    appended to V (PV output rows 64:128). Diagonal k-tiles stream only the
    valid q-column suffix (triangle streaming) and use a single 128x128 mask.
  - Head->sequence resharding via FOUR pipelined AllToAlls (q-blocks 0-3,
    4-5, 6, 7) with a 64-column sub-slice shard layout: q-block j splits into
    8x64 rows, piece d -> core d, so every core receives rows from every
    block; the first three collectives hide under attention and the only
    unhidden one is the single-block collective after q-block 7.
  - Output projection + bias (rank-1 ones trick) + LayerNorm per 128-row pair
    of blocks. Attention q-blocks 6-7 are exp(ACT)-paced, so NO Wo work rides
    inline there: the three ready pairs are emitted after q-block 7's bounce
    and execute warm on the otherwise-idle PE inside the final collective
    window. In-stream pairs compute rsqrt(var+eps) on DVE (bit-trick seed +
    2 Newton steps) so ACT keeps the Exp table; the tail pair uses ACT Sqrt +
    reciprocal since no exps remain and its table switch preloads for free
    during the collective.
Host gathers the 8 per-core interleaved row-slices into the full output.
"""
import os
import sys

sys.path.insert(0, "/opt/trn_rl_repo")

import numpy as np
import ml_dtypes

BF = ml_dtypes.bfloat16

DIM = 1024
H = 16
NCORES = 8
QB = 512          # query block (columns of transposed scores)
KT = 128          # key tile (partition dim of transposed scores)
NDIAG = QB // KT  # k-tiles crossing the causal diagonal per q block
SUB = QB // NCORES  # 64: per-core sub-slice of a q block after resharding
LN_EPS = 1e-5
ROPE_THETA = 10000.0


_built = {}


def _build(S, triv_ln=False):
    """Build + compile the 8-core SPMD graph for sequence length S."""
    import concourse.bass as bass
    import concourse.bacc as bacc
    import concourse.tile as tile
    import concourse.mybir as mybir

    f32 = mybir.dt.float32
    bf16 = mybir.dt.bfloat16
    AF = mybir.ActivationFunctionType
    OP = mybir.AluOpType

    assert S % 512 == 0 and (S // NCORES) % 64 == 0
    SLC = S // NCORES          # output rows per core
    NQB = S // QB              # number of query blocks (= chunks)
    NKT = S // KT              # number of key tiles
    NCH = S // 512             # 512-wide chunks for projections
    NPAIR = NQB // 2           # 128-row Wo/LN tiles (pairs of blocks)
    assert NQB % 2 == 0

    # collective split: groups of q-blocks per AllToAll; pairs early so Wo
    # work spreads across ACT-bound attention blocks, last blocks in their
    # own small collectives so the unhidable tail is minimal
    if NQB >= 6:
        a2a_groups = [list(range(0, NQB - 4)),
                      [NQB - 4, NQB - 3],
                      [NQB - 2], [NQB - 1]]
    elif NQB > 2:
        a2a_groups = [list(range(0, NQB - 2)), [NQB - 2, NQB - 1]]
    else:
        a2a_groups = [list(range(NQB))]
    # qb -> (group index, slot within group)
    qb_group = {}
    for gi, grp in enumerate(a2a_groups):
        for si, j in enumerate(grp):
            qb_group[j] = (gi, si)

    nc = bacc.Bacc("TRN2", target_bir_lowering=False, debug=False, num_devices=NCORES)

    xt_d = nc.dram_tensor("xt", [128, S // 512, 8, 512], bf16, kind="ExternalInput").ap()
    wqkv_d = nc.dram_tensor("wqkv", [128, 8, 3, 128], bf16, kind="ExternalInput").ap()
    b3_d = nc.dram_tensor("b3", [128, 3], f32, kind="ExternalInput").ap()
    bv_d = nc.dram_tensor("bv", [128, 128], f32, kind="ExternalInput").ap()
    cs_d = nc.dram_tensor("cs", [128, 2, S], bf16, kind="ExternalInput").ap()
    msk_d = nc.dram_tensor("msk", [128, 128], bf16, kind="ExternalInput").ap()
    wo_d = nc.dram_tensor("wo", [128, 8, DIM], bf16, kind="ExternalInput").ap()
    bo16_d = nc.dram_tensor("bo16", [1, DIM], bf16, kind="ExternalInput").ap()
    lnc_d = nc.dram_tensor("lnc", [128, 2, DIM], f32, kind="ExternalInput").ap()
    out_d = nc.dram_tensor("out", [SLC, DIM], f32, kind="ExternalOutput").ap()

    with tile.TileContext(nc) as tc:
        with (
            tc.tile_pool(name="const", bufs=1) as cp,
            tc.tile_pool(name="dram", bufs=1, space="DRAM") as dramp,
        ):
            wqkv = cp.tile([128, 8, 3, 128], bf16)
            b3 = cp.tile([128, 3], f32)
            bv = cp.tile([128, 128], f32)
            cs = cp.tile([128, 2, S], bf16)
            msk = cp.tile([128, 128], bf16)
            wo = cp.tile([128, 8, DIM], bf16)
            bo16 = cp.tile([1, DIM], bf16)
            q_sbs = [cp.tile([128, 512], bf16, name=f"qsb{c}") for c in range(NCH)]
            k_sbs = [cp.tile([128, 512], bf16, name=f"ksb{c}") for c in range(NCH)]
            # [V_A(64) | ones(64) | V_B(64) | ones(64)] per k-tile: the ones block
            # makes the PV matmul emit the softmax denominator broadcast across
            # partitions 64:128 of the ctx accumulator.
            v_alls = [cp.tile([128, 256], bf16, name=f"vall{s}") for s in range(NKT)]
            ctxTs = [cp.tile([128, QB], bf16, name=f"ctxT{j}") for j in range(NQB)]
            # received (post-a2a) context: [part, src-core, block, sub-col]
            ctxF = cp.tile([128, NCORES, NQB, SUB], bf16)
            if not triv_ln:
                lnc = cp.tile([128, 2, DIM], f32)

            epsc = cp.tile([128, 1], f32)
            nc.vector.memset(epsc[:], LN_EPS)
            ones1 = cp.tile([1, 128], bf16)
            nc.vector.memset(ones1[:], 1.0)

            # a2a staging (internal DRAM)
            a2a_ins = [dramp.tile([NCORES, 128, len(g) * SUB], bf16,
                                  name=f"a2ain{i}") for i, g in enumerate(a2a_groups)]
            a2a_outs = [dramp.tile([NCORES, 128, len(g) * SUB], bf16,
                                   name=f"a2aout{i}") for i, g in enumerate(a2a_groups)]

            with (
                tc.tile_pool(name="p1", bufs=1) as p1,
                tc.tile_pool(name="p2", bufs=1) as p2,
                tc.tile_pool(name="ps_mix", bufs=3, space="PSUM") as psmix,
                tc.tile_pool(name="ps_ctx", bufs=1, space="PSUM") as psctx,
            ):
                def load_chunk(ch, eng=None, split=1):
                    xtc = p1.tile([128, 8, 512], bf16, tag="xtc", bufs=3)
                    for s_ in range(split):
                        t0, t1 = 8 * s_ // split, 8 * (s_ + 1) // split
                        (eng or nc.sync).dma_start(
                            xtc[:, t0:t1, :], xt_d[:, ch, t0:t1, :])
                    return xtc

                # ── single-issue staged loads. DMAs serialize on their issuing
                # engine, so SP carries only the critical path (weights + x
                # chunks; chunk 0 rides the idle ACT queue so it overlaps
                # wqkv) and the bulky/early-idle loads go via Pool. cs is
                # split so chunk 0's slice lands fast (deps are AP-granular) ──
                nc.sync.dma_start(wqkv[:, 0:4, :, :], wqkv_d[:, 0:4, :, :])
                nc.sync.dma_start(wqkv[:, 4:8, :, :], wqkv_d[:, 4:8, :, :])
                xtcs = {0: load_chunk(0, eng=nc.scalar, split=2)}
                nc.sync.dma_start(b3[:], b3_d[:])
                nc.sync.dma_start(cs[:, :, 0:512], cs_d[:, :, 0:512])
                nc.gpsimd.dma_start(bv[:], bv_d[:])
                nc.gpsimd.dma_start(msk[:], msk_d[:])
                nc.gpsimd.dma_start(cs[:, :, 512:S], cs_d[:, :, 512:S])
                nc.gpsimd.dma_start(bo16[:], bo16_d[:])
                nc.gpsimd.dma_start(wo[:], wo_d[:])
                if not triv_ln:
                    nc.gpsimd.dma_start(lnc[:], lnc_d[:])

                def proj_mm(j3, xtc):
                    ps = psmix.tile([128, 512], f32, tag="sc",
                                    padded_shape=[128, 2 * QB], name="psproj")
                    for t in range(8):
                        nc.tensor.matmul(
                            ps[:], wqkv[:, t, j3, :], xtc[:, t, :],
                            start=(t == 0), stop=(t == 7))
                    return ps

                def rope_drain(j3, ps, dst, ch, mA, mB, sl):
                    # rot = (p+b)*cos_dup + (p_swapped+b)*sin_signed over cols sl
                    bq0 = b3[0:64, j3:j3 + 1]
                    bq1 = b3[64:128, j3:j3 + 1]
                    csl = cs[:, :, 512 * ch + sl.start:512 * ch + sl.stop]
                    nc.vector.scalar_tensor_tensor(
                        mA[:, sl], ps[:, sl], b3[:, j3:j3 + 1], csl[:, 0, :],
                        op0=OP.add, op1=OP.mult)
                    nc.vector.scalar_tensor_tensor(
                        mB[0:64, sl], ps[64:128, sl], bq1, csl[64:128, 1, :],
                        op0=OP.add, op1=OP.mult)
                    nc.vector.scalar_tensor_tensor(
                        mB[64:128, sl], ps[0:64, sl], bq0, csl[0:64, 1, :],
                        op0=OP.add, op1=OP.mult)
                    nc.vector.tensor_add(dst[:, sl], mA[:, sl], mB[:, sl])

                def proj_chunk(j3, dst, ch, xtc, nsplit=1):
                    # q/k projection for 512 seq cols in [ch, seq] layout,
                    # RoPE + bias fused on the PSUM drain. nsplit>1 drains in
                    # column pieces so the first consumer can start earlier.
                    ps = proj_mm(j3, xtc)
                    mA = p1.tile([128, 512], f32, tag="mA", bufs=2)
                    mB = p1.tile([128, 512], f32, tag="mB", bufs=2)
                    for s_ in range(nsplit):
                        sl = slice(512 * s_ // nsplit, 512 * (s_ + 1) // nsplit)
                        rope_drain(j3, ps, dst, ch, mA, mB, sl)

                def proj_v_chunk(ch, xtc):
                    # V projected directly transposed ([kpos, ch] via 128-col
                    # matmuls) — no PE transposes, one 1-bank PSUM tile, bias
                    # added on the drain into the v_alls interleaved layout
                    vpt = psmix.tile([128, 4, 128], f32, tag="sc",
                                     padded_shape=[128, 4, QB // 2], name="psvt")
                    for j in range(4):
                        for t in range(8):
                            nc.tensor.matmul(
                                vpt[:, j, :], xtc[:, t, 128 * j:128 * (j + 1)],
                                wqkv[:, t, 2, :], start=(t == 0), stop=(t == 7))
                    for j in range(4):
                        st = 4 * ch + j
                        meng = nc.gpsimd if ch >= 2 else nc.vector
                        meng.memset(
                            v_alls[st][:].rearrange("p (g c) -> p g c", c=64)[:, 1:4:2, :],
                            1.0)
                        nc.vector.tensor_tensor(
                            v_alls[st][:].rearrange("p (g c) -> p g c", c=64)[:, 0:4:2, :],
                            vpt[:, j, :].rearrange("p (g c) -> p g c", c=64),
                            bv[:].rearrange("p (g c) -> p g c", c=64),
                            op=OP.add)

                def emit_qk(qb, kt):
                    # one group = one 128-wide k-tile against the valid part of
                    # the 512-wide q block; head A scores land in bank 0 of the
                    # sc slot, head B in bank 1. Diagonal k-tiles (r>=1) only
                    # stream the q-column suffix that is not fully masked.
                    r = kt - NDIAG * qb
                    c0 = 128 * r if r > 0 else 0
                    kch, ko = kt // 4, 128 * (kt % 4)
                    sc = psmix.tile([128, 2, QB], f32, tag="sc",
                                    padded_shape=[128, 2, QB], name="scsc")
                    pt = p2.tile([128, 2, QB], bf16, tag="pt", bufs=4)
                    npc = 1
                    for p_ in range(npc):
                        pl = slice(c0 + (QB - c0) * p_ // npc,
                                   c0 + (QB - c0) * (p_ + 1) // npc)
                        nc.tensor.matmul(
                            sc[:, 0, pl], k_sbs[kch][0:64, ko:ko + 128],
                            q_sbs[qb][0:64, pl], start=True, stop=True)
                        nc.tensor.matmul(
                            sc[:, 1, pl], k_sbs[kch][64:128, ko:ko + 128],
                            q_sbs[qb][64:128, pl], start=True, stop=True)
                        nc.scalar.activation(pt[:, :, pl], sc[:, :, pl],
                                             AF.Exp, scale=0.125)
                    return pt

                def emit_pv(qb, kt, first, last, pt, ctx):
                    nk = (QB * (qb + 1)) // KT
                    r = kt - NDIAG * qb
                    c0 = 128 * r if r > 0 else 0
                    if r >= 0:  # mask the single diagonal-crossing 128-col band
                        nc.vector.tensor_mul(
                            pt[:, 0, c0:c0 + 128], pt[:, 0, c0:c0 + 128], msk[:])
                        nc.vector.tensor_mul(
                            pt[:, 1, c0:c0 + 128], pt[:, 1, c0:c0 + 128], msk[:])
                    nc.tensor.matmul(
                        ctx[:, 0, c0:], v_alls[kt][:, 0:128], pt[:, 0, c0:],
                        start=(kt == 0), stop=(kt == nk - 1))
                    nc.tensor.matmul(
                        ctx[:, 1, c0:], v_alls[kt][:, 128:256], pt[:, 1, c0:],
                        start=(kt == 0), stop=(kt == nk - 1))

                def emit_norm(qb, ctx):
                    # softmax normalize (only one DVE input may be PSUM, so
                    # reciprocal denominators into SBUF first, then 2 muls).
                    dst = ctxTs[qb]
                    rb = p2.tile([64, 2, QB], f32, tag="rb", bufs=2)
                    nc.vector.reciprocal(rb[:], ctx[64:128, :, :])
                    nc.vector.tensor_mul(dst[0:64, :], ctx[0:64, 0, :], rb[:, 0, :])
                    nc.vector.tensor_mul(dst[64:128, :], ctx[0:64, 1, :], rb[:, 1, :])
                    # bounce the 8 per-core 64-col sub-slices into a2a staging
                    # (DRAM-side AP permuted so the SBUF side stays partition-first)
                    gi, si = qb_group[qb]
                    nc.gpsimd.dma_start(
                        a2a_ins[gi][:, :, SUB * si:SUB * (si + 1)]
                        .rearrange("d p c -> p d c"),
                        dst[:].rearrange("p (d c) -> p d c", d=NCORES))

                def emit_collective(gi):
                    nc.gpsimd.collective_compute(
                        "AllToAll",
                        mybir.AluOpType.bypass,
                        replica_groups=[list(range(NCORES))],
                        ins=[a2a_ins[gi][:].opt()],
                        outs=[a2a_outs[gi][:].opt()],
                    )
                    # receive: [src, 128, blocks*SUB] -> [128, src, blocks, SUB]
                    # split by src half so the Wo ct-loop can start on the
                    # first half while the second is still landing
                    j0 = a2a_groups[gi][0]
                    nblk = len(a2a_groups[gi])
                    reng = nc.gpsimd if gi == len(a2a_groups) - 1 else nc.sync
                    for h in range(2):
                        ds = slice(4 * h, 4 * (h + 1))
                        reng.dma_start(
                            ctxF[:, ds, j0:j0 + nblk, :],
                            a2a_outs[gi][ds]
                            .rearrange("d p (j c) -> p d j c", c=SUB))

                def emit_wo_pair(t, tail=False):
                    # Wo + bias + LayerNorm for 128 output rows: blocks 2t,2t+1
                    ops = psmix.tile([128, DIM], f32, tag="sc",
                                     padded_shape=[128, 2 * QB], name="pswo")
                    stats = p2.tile([128, 2, 6], f32, tag="stats", bufs=2)
                    for nch in range(DIM // 512):
                        osl = slice(512 * nch, 512 * (nch + 1))
                        for ct in range(8):
                            nc.tensor.matmul(
                                ops[:, osl], ctxF[:, ct, 2 * t:2 * t + 2, :],
                                wo[:, ct, osl], start=(ct == 0), stop=False)
                        # rank-1 bias add closes the PSUM group
                        nc.tensor.matmul(
                            ops[:, osl], ones1[:], bo16[:, osl],
                            start=False, stop=True)
                        # per-half stats overlap the other half's matmuls
                        nc.vector.bn_stats(stats[:, nch, :], ops[:, osl])
                    mv = p2.tile([128, 2], f32, tag="mv", bufs=2)
                    nc.vector.bn_aggr(mv[:], stats[:])
                    # 1/sqrt(var+eps): DVE-only bit-trick (so ACT keeps the
                    # Exp table) for in-stream pairs; the tail pair uses ACT
                    # Rsqrt — no exps remain, and its table switch preloads
                    # during the final collective window.
                    i32 = mybir.dt.int32
                    sd = p2.tile([128, 6], f32, tag="sd", bufs=2)
                    if tail:
                        nc.scalar.activation(sd[:, 1:2], mv[:, 1:2], AF.Sqrt,
                                             bias=epsc[:])
                        y = sd[:, 0:1]
                        nc.vector.reciprocal(y, sd[:, 1:2])
                    else:
                        sdi = p2.tile([128, 1], i32, tag="sdi", bufs=2)
                        xe = sd[:, 0:1]
                        nc.vector.tensor_scalar_add(xe, mv[:, 1:2], epsc[:])
                        nc.vector.tensor_scalar(
                            sdi[:], xe.bitcast(i32), 1, None,
                            op0=OP.arith_shift_right)
                        nc.vector.tensor_scalar(
                            sdi[:], sdi[:], -1, 0x5F3759DF, op0=OP.mult, op1=OP.add)
                        y = sdi[:].bitcast(f32)
                        for _ in range(2):
                            a = sd[:, 1:2]
                            nc.vector.tensor_mul(a, y, y)
                            nc.vector.tensor_mul(a, a, xe)
                            nc.vector.tensor_scalar(
                                a, a, -0.5, 1.5, op0=OP.mult, op1=OP.add)
                            nc.vector.tensor_mul(y, y, a)
                    t2 = p2.tile([128, DIM], f32, tag="t2", bufs=2)
                    tsl = slice(128 * t, 128 * (t + 1))
                    if triv_ln:
                        # normalize + store in halves so the DMA overlaps DVE
                        for nch in range(2):
                            osl = slice(512 * nch, 512 * (nch + 1))
                            nc.vector.tensor_scalar(
                                t2[:, osl], ops[:, osl], mv[:, 0:1], y,
                                op0=OP.subtract, op1=OP.mult)
                            nc.sync.dma_start(out_d[tsl, osl], t2[:, osl])
                    else:
                        nc.vector.tensor_scalar(
                            t2[:], ops[:], mv[:, 0:1], y,
                            op0=OP.subtract, op1=OP.mult)
                        t3 = p2.tile([128, DIM], f32, tag="t3", bufs=2)
                        nc.vector.tensor_mul(t3[:], t2[:], lnc[:, 0, :])
                        ob = p2.tile([128, DIM], f32, tag="ob", bufs=2)
                        nc.vector.tensor_add(ob[:], t3[:], lnc[:, 1, :])
                        nc.sync.dma_start(out_d[tsl, :], ob[:])

                # ───────── streamed projection + attention, interleaved ─────
                from collections import deque
                pending = deque()   # (qb, kt, first, last, pt, ctx)
                ctx = None

                def attn_group(qb, kt, nk):
                    nonlocal ctx
                    if kt == 0:
                        ctx = psctx.tile([128, 2, QB], f32, tag="ctx",
                                         padded_shape=[128, 2, QB])
                    pt = emit_qk(qb, kt)
                    pending.append((qb, kt, kt == 0, kt == nk - 1, pt, ctx))
                    if len(pending) > 3:
                        d = pending.popleft()
                        emit_pv(*d)
                        if d[3]:
                            emit_norm(d[0], d[5])

                # chunk 0 projected up front; chunk 0's K-rope drains in
                # 128-col pieces so QK(0,0) starts as early as possible.
                proj_chunk(0, q_sbs[0], 0, xtcs[0])
                proj_chunk(1, k_sbs[0], 0, xtcs[0], nsplit=4)
                proj_v_chunk(0, xtcs[0])

                # Wo pairs scheduled into later q-blocks' attention streams:
                # pair t may start once its a2a group has completed; groups
                # fire after q-blocks 3, 5, 7 (for NQB=8). Place pairs
                # (0,1) in qb 6 and (2,3),(4,5) in qb 7 with spacing.
                wo_sched = {}
                if NQB >= 6:
                    # only pair 0 rides inline (qb6 has ACT slack for it);
                    # pairs 1,2 are emitted after qb7's bounce so they run on
                    # the otherwise-idle PE during the last collective window
                    wo_sched = {}
                    npair_tail = NPAIR
                elif NQB > 2:
                    wo_sched = {NQB - 1: {4: 0}}
                    npair_tail = NPAIR - 1
                else:
                    npair_tail = NPAIR

                for qb in range(NQB):
                    nk = (QB * (qb + 1)) // KT
                    # projection pieces for the next chunk, spread over this
                    # q-block's attention groups (q first: needed soonest)
                    pieces = []
                    if qb + 1 < NCH:
                        nxt = qb + 1
                        pieces = [
                            lambda n=nxt: xtcs.__setitem__(n, load_chunk(n)),
                            lambda n=nxt: proj_chunk(0, q_sbs[n], n, xtcs[n]),
                            lambda n=nxt: proj_chunk(1, k_sbs[n], n, xtcs[n]),
                            lambda n=nxt: proj_v_chunk(n, xtcs[n]),
                        ]
                    # positions: pieces spread mid-block, avoiding kt=0 (the
                    # boundary is already slot-constrained) and the wo slots
                    pos = {}
                    for i, pc in enumerate(pieces):
                        pos.setdefault(((i + 1) * nk) // (len(pieces) + 1),
                                       []).append(pc)
                    wo_here = wo_sched.get(qb, {})
                    for kt in range(nk):
                        attn_group(qb, kt, nk)
                        for pc in pos.get(kt, []):
                            pc()
                        if kt in wo_here and wo_here[kt] < NPAIR - npair_tail:
                            emit_wo_pair(wo_here[kt])
                    # fire this block's a2a if it closes a group — but its
                    # last pv/norm may still be pending; drain first.
                    gi, si = qb_group[qb]
                    if si == len(a2a_groups[gi]) - 1:
                        while pending and pending[0][0] <= qb:
                            d = pending.popleft()
                            emit_pv(*d)
                            if d[3]:
                                emit_norm(d[0], d[5])
                        emit_collective(gi)
                while pending:
                    d = pending.popleft()
                    emit_pv(*d)
                    if d[3]:
                        emit_norm(d[0], d[5])
                for t in range(NPAIR - npair_tail, NPAIR):
                    emit_wo_pair(t, tail=(t == NPAIR - 1))

    nc.compile()
    return nc


def get_nc(S=4096, triv_ln=False):
    key = (S, triv_ln)
    if key not in _built:
        _built[key] = _build(S, triv_ln)
    return _built[key]


def stage_inputs(x, Wqkv, bqkv, Wo, bo, gamma, beta):
    """Host-side sharding/staging. Returns in_maps for the 8 cores."""
    x = np.asarray(x, dtype=np.float32)
    Wqkv = np.asarray(Wqkv, dtype=np.float32)
    bqkv = np.asarray(bqkv, dtype=np.float32)
    Wo = np.asarray(Wo, dtype=np.float32)
    bo = np.asarray(bo, dtype=np.float32)
    gamma = np.asarray(gamma, dtype=np.float32)
    beta = np.asarray(beta, dtype=np.float32)

    S = x.shape[1]
    # xt_sw[p, c, t, s] = x[0][512c+s, 128t+p]: chunk-major so each chunk
    # load is one contiguous 8KB-per-partition DMA run
    xt_sw = np.ascontiguousarray(
        x[0].T.reshape(8, 128, S // 512, 512).transpose(1, 2, 0, 3)).astype(BF)
    inv_freq = 1.0 / (ROPE_THETA ** (np.arange(0, DIM, 2, dtype=np.float64) / DIM))

    # Wo rows permuted to the post-AllToAll channel order
    perm = np.concatenate([
        np.concatenate([np.arange(64 * j, 64 * j + 64),
                        np.arange(512 + 64 * j, 512 + 64 * j + 64)])
        for j in range(NCORES)
    ])
    wo_sw = np.ascontiguousarray(
        Wo[perm, :].reshape(8, 128, DIM).transpose(1, 0, 2)).astype(BF)

    p = np.arange(128)[:, None]
    f = np.arange(128)[None, :]
    msk = (p <= f).astype(BF)                  # single diagonal-band mask
    lnc = np.stack([
        np.broadcast_to(gamma, (128, DIM)),
        np.broadcast_to(beta, (128, DIM)),
    ], axis=1).astype(np.float32)              # [128, 2, DIM]

    in_maps = []
    for c in range(NCORES):
        cols = np.concatenate([np.arange(64 * c, 64 * c + 64),
                               np.arange(512 + 64 * c, 512 + 64 * c + 64)])
        ang = np.arange(S, dtype=np.float64)[None, :] * inv_freq[64 * c:64 * c + 64][:, None]
        C = np.cos(ang)
        Sn = np.sin(ang)
        # plane 0: cos duplicated; plane 1: +sin rows 0:64, -sin rows 64:128
        # (the sign flip folds the rotate-half subtraction into one tensor_add)
        cs = np.stack([np.concatenate([C, C], 0),
                       np.concatenate([Sn, -Sn], 0)], axis=1).astype(BF)  # [128,2,S]
        # wqkv_sw[p, t, j, ch] = Wqkv[128t+p, 1024j + cols[ch]]
        wq3 = np.stack([Wqkv[:, 1024 * j + cols] for j in range(3)], axis=1)
        wqkv_sw = np.ascontiguousarray(
            wq3.reshape(8, 128, 3, 128).transpose(1, 0, 2, 3)).astype(BF)
        b3 = np.stack([bqkv[cols], bqkv[1024 + cols], bqkv[2048 + cols]],
                      axis=1).astype(np.float32)             # [128, 3]
        bv = np.broadcast_to(
            bqkv[2048 + cols], (128, 128)).astype(np.float32).copy()
        in_maps.append({
            "xt": xt_sw,
            "wqkv": wqkv_sw,
            "b3": b3,
            "bv": bv,
            "cs": cs,
            "msk": msk,
            "wo": wo_sw,
            "bo16": bo.reshape(1, DIM).astype(BF),
            "lnc": lnc,
        })
    return in_maps


def unshard_output(res_list, S):
    """res_list[c] = [SLC, DIM] with rows [64j+r] = full rows 512j+64c+r."""
    NQB = S // QB
    R = np.stack(res_list)                       # [8, NQB*64, DIM]
    R = R.reshape(NCORES, NQB, SUB, DIM).transpose(1, 0, 2, 3)
    return R.reshape(S, DIM)


def kernel(x, Wqkv, bqkv, Wo, bo, gamma, beta):
    from concourse import bass_utils

    x = np.asarray(x)
    S = x.shape[1]
    triv = bool(np.all(np.asarray(gamma) == 1.0) and np.all(np.asarray(beta) == 0.0))
    nc = get_nc(S, triv)
    in_maps = stage_inputs(x, Wqkv, bqkv, Wo, bo, gamma, beta)
    res = bass_utils.run_bass_kernel_spmd(nc, in_maps, core_ids=list(range(NCORES)))
    out = unshard_output([res.results[c]["out"] for c in range(NCORES)], S)
    return out[None].astype(np.float32)
